# revision 16
# baseline (speedup 1.0000x reference)
"""Multi-head self-attention TRN2 Bass kernel (8 NeuronCores).

Sharding: core c -> batch b = c//2, head-group g = c%2 (8 of 16 heads).
Data-parallel over batch, tensor-parallel over heads; the two cores
sharing a batch produce partial output projections that the host sums
(the all-reduce of the output projection, folded into unsharding).

All matmul operands are bf16 (fp32r streams at half rate on TRN2 HW;
bf16 runs the PE at full rate, ~216ns per [128x128]x[128,512] matmul).
PSUM accumulation stays fp32; exp runs on ScalarE from PSUM.

Per-core device program:
  QK(hp0): Q^T,K^T = Wq/Wk^T.T @ x^T per head pair; Q stored zero-padded
           per head (qzA rows 64:128 = 0, qzB rows 0:64 = 0) so the d=64
           scores contraction runs as K=128.
  V      : V = x @ Wv in natural [seq, feat] layout (+ ones cols -> sums)
  attention(hp) per 512-query block, two k-chunks per step:
     S^T[k, q] = K2^T.T @ qz               (PSUM [128,1024])
     P^T = exp(S^T/8)                      (ScalarE, bf16 out)
     O_aug^T += V_pad.T @ P^T              (row 64 = softmax denominators)
     O^T = O_aug^T[0:64] * approx_recip(sums)  (DVE + GpSimd bcast)
  QK(hp+1) is interleaved into attention(hp) (qz/k2 double-buffered) so
  the PE fills the exp-wait slack; the output projection is fused into
  attention(hp3) per query block (delayed by one) to kill the tail.
"""
import numpy as np
from contextlib import ExitStack

import ml_dtypes

import concourse.bass as bass
import concourse.mybir as mybir
import concourse.tile as tile
from concourse import bacc
from concourse.bass_utils import run_bass_kernel_spmd

f32 = mybir.dt.float32
bf16 = mybir.dt.bfloat16
EXP = mybir.ActivationFunctionType.Exp
MULT = mybir.AluOpType.mult

B, N = 4, 2048
DIM = 1024
HL = 8          # heads per core
DH = 64
KD = DIM // 128  # 8 contraction chunks
NS = N // 128    # 16 key chunks
NQ = N // 512    # 4 query blocks
HP = HL // 2     # 4 head pairs


def build():
    nc = bacc.Bacc(None, target_bir_lowering=False, debug=False)
    xT = nc.declare_dram_parameter("xT", [DIM, N], bf16, isOutput=False)
    wq = nc.declare_dram_parameter("wq", [DIM, HL * DH], bf16, isOutput=False)
    wk = nc.declare_dram_parameter("wk", [DIM, HL * DH], bf16, isOutput=False)
    wv = nc.declare_dram_parameter("wv", [DIM, HL * DH], bf16, isOutput=False)
    wo = nc.declare_dram_parameter("wo", [HL * DH, DIM], bf16, isOutput=False)
    yT = nc.declare_dram_parameter("yT", [DIM, N], f32, isOutput=True)

    with tile.TileContext(nc) as tc, ExitStack() as ctx:
        p1 = ctx.enter_context(tc.tile_pool(name="p1", bufs=1))
        p2 = ctx.enter_context(tc.tile_pool(name="p2", bufs=2))
        psS = ctx.enter_context(tc.tile_pool(name="psS", bufs=2, space="PSUM"))
        psV = ctx.enter_context(tc.tile_pool(name="psV", bufs=4, space="PSUM"))

        # ---- parameter loads, spread across four engine DMA queues so the
        # transfers overlap (sync: x; scalar: wq/wk; vector: wv; gpsimd: wo)
        wqt, wkt, xt = [], [], []
        for k in range(KD):
            tq = p1.tile([128, HL * DH], bf16, tag=f"wq{k}", name=f"wq{k}")
            nc.scalar.dma_start(out=tq[:], in_=wq[k * 128:(k + 1) * 128, :])
            wqt.append(tq)
            tk = p1.tile([128, HL * DH], bf16, tag=f"wk{k}", name=f"wk{k}")
            nc.scalar.dma_start(out=tk[:], in_=wk[k * 128:(k + 1) * 128, :])
            wkt.append(tk)
            t = p1.tile([128, N], bf16, tag=f"xt{k}", name=f"xt{k}")
            nc.sync.dma_start(out=t[:], in_=xT[k * 128:(k + 1) * 128, :])
            xt.append(t)
        wvt = []
        for k in range(KD):
            tw = p1.tile([128, HL * DH], bf16, tag=f"wv{k}", name=f"wv{k}")
            nc.gpsimd.dma_start(out=tw[:], in_=wv[k * 128:(k + 1) * 128, :])
            wvt.append(tw)
        wot = []
        for j in range(HP):
            tw = p1.tile([128, DIM], bf16, tag=f"wo{j}", name=f"wo{j}")
            nc.gpsimd.dma_start(out=tw[:], in_=wo[j * 128:(j + 1) * 128, :])
            wot.append(tw)

        # ---- persistent attention tiles ----
        # qz/k2 double-buffered across head pairs
        qz = [[p1.tile([128, N], bf16, tag=f"qz{h}_{b}", name=f"qz{h}_{b}")
               for h in range(2)] for b in range(2)]
        k2 = [p1.tile([128, N], bf16, tag=f"k2_{b}", name=f"k2_{b}")
              for b in range(2)]
        for b in range(2):
            nc.vector.memset(qz[b][0][64:128, :].bitcast(f32), 0.0)
            nc.vector.memset(qz[b][1][0:64, :].bitcast(f32), 0.0)

        # per-head stride 66 (not 65) keeps every head's 128-col lhsT
        # window 4-byte aligned in bf16; cols 64 AND 65 are ones (col 64 is
        # the denominator row, 65 is junk M-row padding anyway)
        v2 = [p1.tile([128, HL * 66 + 64], bf16, tag=f"v2_{st}", name=f"v2_{st}")
              for st in range(NS)]
        ot = [p1.tile([128, N], bf16, tag=f"ot{j}", name=f"ot{j}")
              for j in range(HP)]

        def emit_qk(hp):
            """project Q^T and K^T for head pair hp into qz/k2 buffer hp%2;
            yields 4 times (once per quarter) so the caller can interleave."""
            buf = hp % 2
            for which, warr in (("q", wqt), ("k", wkt)):
                for nb in ((0, 1), (2, 3)):
                    pss = psS.tile([128, 1024], f32, tag="s", name="qkps")
                    for k in range(KD):
                        lhsT = warr[k][:, hp * 128:(hp + 1) * 128]
                        for i, n in enumerate(nb):
                            nc.tensor.matmul(
                                pss[:, i * 512:(i + 1) * 512], lhsT,
                                xt[k][:, n * 512:(n + 1) * 512],
                                start=(k == 0), stop=(k == KD - 1))
                    nsl = slice(nb[0] * 512, (nb[1] + 1) * 512)
                    if which == "q":
                        nc.vector.tensor_copy(out=qz[buf][0][0:64, nsl],
                                              in_=pss[0:64, :])
                        nc.vector.tensor_copy(out=qz[buf][1][64:128, nsl],
                                              in_=pss[64:128, :])
                    else:
                        nc.vector.tensor_copy(out=k2[buf][:, nsl], in_=pss[:, :])
                    yield

        # f32 whose bits are two packed bf16 1.0s (for the paired ones cols)
        ONES2 = float(np.frombuffer(b"\x80\x3f\x80\x3f", "<f4")[0])

        def emit_v_pair(st):
            """project V chunks st, st+1 into one [128,1024] psS-pool tile
            (scores tag: rotates acyclically with the s tiles, unlike the
            pv tag whose buffers are held across a whole query block)."""
            vps = psS.tile([128, 1024], f32, tag="s", name="vps2")
            for half in range(2):
                v2t = v2[st + half]
                v3 = v2t[:, 0:HL * 66].rearrange("p (h c) -> p h c", h=HL)
                ones = v2t[:, 0:HL * 66].bitcast(f32).rearrange(
                    "p (h c) -> p h c", h=HL)  # [128, 8, 33] f32 view
                nc.vector.memset(ones[:, :, 32:33], ONES2)
                nc.vector.memset(v2t[:, HL * 66:].bitcast(f32), 0.0)
                vh = vps[:, half * 512:(half + 1) * 512]
                for k in range(KD):
                    nc.tensor.matmul(
                        vh, xt[k][:, (st + half) * 128:(st + half + 1) * 128],
                        wvt[k][:], start=(k == 0), stop=(k == KD - 1))
                nc.vector.tensor_copy(
                    out=v3[:, :, 0:64],
                    in_=vh.rearrange("p (h d) -> p h d", h=HL))

        def emit_outproj(n):
            """output projection for query block n (needs all ot)."""
            for dt in range(KD):
                yps = psV.tile([128, 512], f32, tag="pv", name="yps")
                for j in range(HP):
                    nc.tensor.matmul(yps[:], wot[j][:, dt * 128:(dt + 1) * 128],
                                     ot[j][:, n * 512:(n + 1) * 512],
                                     start=(j == 0), stop=(j == HP - 1))
                ysb = p2.tile([128, 512], f32, tag="y", name="ysb")
                nc.vector.tensor_copy(out=ysb[:], in_=yps[:])
                nc.sync.dma_start(out=yT[dt * 128:(dt + 1) * 128,
                                         n * 512:(n + 1) * 512], in_=ysb[:])

        # Schraudolph exp on the DVE: bf16 bits of exp(x) ~= int16 round of
        # x*(log2e*128) + (127 - sigma)*128, computed as one fused
        # tensor_scalar (mult, add) with int16 convert-on-write, then the
        # int16 tile is bitcast to bf16.  Max rel ripple ~3%; applied to a
        # minority of tiles to offload the saturated ScalarE.
        SCH_A = float(0.125 * np.log2(np.e) * 128.0)
        SCH_B = float((127.0 - 0.0430) * 128.0)
        ADD = mybir.AluOpType.add
        i16 = mybir.dt.int16

        def emit_attention(hp, filler, step_hook=None, dve_steps=()):
            """attention for head pair hp; `filler` is a list of callables
            (one per query block) emitted after each block to fill the PE
            while ScalarE catches up on the exp backlog.  step_hook(qb, ms)
            is emitted inside the ms loop; h01=1 exps of ms-steps listed in
            dve_steps run on the DVE (Schraudolph) instead of ScalarE."""
            buf = hp % 2
            kk, qq = k2[buf], qz[buf]
            for qb in range(NQ):
                qsl = slice(qb * 512, (qb + 1) * 512)
                pv = [psV.tile([128, 512], f32, tag="pv", name="pv")
                      for _ in range(2)]
                sps = {}

                def emit_s2(ms):
                    s = [psS.tile([128, 1024], f32, tag="s", name="s")
                         for _ in range(2)]
                    for i in range(2):     # k-chunk ms+i (shared ldweights)
                        for h01 in range(2):
                            nc.tensor.matmul(
                                s[h01][:, i * 512:(i + 1) * 512],
                                kk[:, (ms + i) * 128:(ms + i + 1) * 128],
                                qq[h01][:, qsl], start=True, stop=True)
                    sps[ms] = s

                emit_s2(0)
                for ms in range(0, NS, 2):
                    if step_hook is not None:
                        step_hook(qb, ms)
                    if ms + 2 < NS:
                        emit_s2(ms + 2)
                    s = sps.pop(ms)
                    for h01 in range(2):
                        if h01 == 1 and (ms // 2) in dve_steps:
                            pti = p2.tile([128, 1024], i16, tag="pti",
                                          name="pti", bufs=2)
                            nc.vector.tensor_scalar(
                                out=pti[:], in0=s[h01][:],
                                scalar1=SCH_A, scalar2=SCH_B,
                                op0=MULT, op1=ADD)
                            pt = pti.bitcast(bf16)
                        else:
                            pt = p2.tile([128, 1024], bf16, tag="pt",
                                         name="pt", bufs=4)
                            nc.scalar.activation(pt[:], s[h01][:], EXP,
                                                 scale=0.125)
                        l = hp * 2 + h01
                        nc.tensor.matmul(pv[h01][:],
                                         v2[ms][:, l * 66:l * 66 + 128],
                                         pt[:, 0:512],
                                         start=(ms == 0), stop=False)
                        nc.tensor.matmul(pv[h01][:],
                                         v2[ms + 1][:, l * 66:l * 66 + 128],
                                         pt[:, 512:1024],
                                         start=False, stop=(ms + 2 == NS))
                # normalize + evict: denominator row to SBUF partition 0,
                # reciprocal, broadcast across partitions, scale the O rows.
                srowt = p1.tile([1, 1024], f32, tag="srow", name="srow", bufs=2)
                rsum = p1.tile([1, 1024], f32, tag="rsum", name="rsum", bufs=2)
                for h01 in range(2):
                    srow = srowt[:, h01 * 512:(h01 + 1) * 512]
                    nc.vector.tensor_copy(out=srow, in_=pv[h01][64:65, :])
                    rs = rsum[:, h01 * 512:(h01 + 1) * 512]
                    nc.vector.reciprocal_approx_fast(out=rs, in_=srow)
                    rb = p1.tile([64, 512], f32, tag=f"rb{h01}", name="rb",
                                 bufs=2)
                    nc.gpsimd.partition_broadcast(rb[:], rs)
                    lo = h01 * 64
                    nc.vector.tensor_tensor(out=ot[hp][lo:lo + 64, qsl],
                                            in0=pv[h01][0:64, :],
                                            in1=rb[:], op=MULT)
                if filler and qb < len(filler):
                    filler[qb]()

        # ---- program order ----
        qk0 = list(emit_qk(0))  # generator fully drained: QK(hp0) up front
        del qk0
        emit_v_pair(0)

        def v_hook(qb, ms):
            # hp0 qb0: V projection chunks just-in-time, two per ms step
            if qb == 0 and ms + 2 < NS:
                emit_v_pair(ms + 2)

        DVE_STEPS = (1, 4, 6)   # 3 of 16 exps per query block on the DVE
        for hp in range(HP):
            if hp + 1 < HP:
                g = emit_qk(hp + 1)
                filler = [lambda g=g: next(g, None) for _ in range(NQ)]
            else:
                # last head pair: fuse the output projection, delayed by
                # one query block so the normalize chain is never waited on
                filler = [lambda n=n: emit_outproj(n) if n >= 0 else None
                          for n in (-1, 0, 1, 2)]
            emit_attention(hp, filler,
                           step_hook=v_hook if hp == 0 else None,
                           dve_steps=DVE_STEPS)
        emit_outproj(NQ - 1)

    nc.finalize()
    return nc


def make_in_map(x_b, w_qkv, w_out, g):
    cols = slice(g * 512, (g + 1) * 512)
    b16 = ml_dtypes.bfloat16
    return {
        "xT": np.ascontiguousarray(x_b.T).astype(b16),
        "wq": np.ascontiguousarray(w_qkv[:, 0:1024][:, cols]).astype(b16),
        "wk": np.ascontiguousarray(w_qkv[:, 1024:2048][:, cols]).astype(b16),
        "wv": np.ascontiguousarray(w_qkv[:, 2048:3072][:, cols]).astype(b16),
        "wo": np.ascontiguousarray(w_out[cols, :]).astype(b16),
    }


_NC_CACHE = {}


def _get_nc():
    if "nc" not in _NC_CACHE:
        _NC_CACHE["nc"] = build()
    return _NC_CACHE["nc"]


def kernel(x, w_qkv, w_out, b_out, trace=False):
    x = np.ascontiguousarray(np.asarray(x, dtype=np.float32))
    w_qkv = np.ascontiguousarray(np.asarray(w_qkv, dtype=np.float32))
    w_out = np.ascontiguousarray(np.asarray(w_out, dtype=np.float32))
    b_out = np.asarray(b_out, dtype=np.float32)

    nc = _get_nc()
    in_maps = [make_in_map(x[c // 2], w_qkv, w_out, c % 2) for c in range(8)]
    r = run_bass_kernel_spmd(nc, in_maps, list(range(8)), trace=trace)
    _NC_CACHE["exec_time_ns"] = r.exec_time_ns

    out = np.empty((B, N, DIM), np.float32)
    for b in range(B):
        out[b] = (r.results[2 * b]["yT"] + r.results[2 * b + 1]["yT"]).T + b_out
    return out


# revision 21
# speedup vs baseline: 1.0315x; 1.0315x over previous
"""Multi-head self-attention TRN2 Bass kernel (8 NeuronCores).

Sharding: core c -> batch b = c//2, head-group g = c%2 (8 of 16 heads).
Data-parallel over batch, tensor-parallel over heads; the two cores
sharing a batch produce partial output projections that the host sums
(the all-reduce of the output projection, folded into unsharding).

All matmul operands are bf16 (fp32r streams at half rate on TRN2 HW;
bf16 runs the PE at full rate, ~216ns per [128x128]x[128,512] matmul).
PSUM accumulation stays fp32; exp runs on ScalarE from PSUM.

Per-core device program:
  QK(hp0): Q^T,K^T = Wq/Wk^T.T @ x^T per head pair; Q stored zero-padded
           per head (qzA rows 64:128 = 0, qzB rows 0:64 = 0) so the d=64
           scores contraction runs as K=128.
  V      : V = x @ Wv in natural [seq, feat] layout (+ ones cols -> sums)
  attention(hp) per 512-query block, two k-chunks per step:
     S^T[k, q] = K2^T.T @ qz               (PSUM [128,1024])
     P^T = exp(S^T/8)                      (ScalarE, bf16 out)
     O_aug^T += V_pad.T @ P^T              (row 64 = softmax denominators)
     O^T = O_aug^T[0:64] * approx_recip(sums)  (DVE + GpSimd bcast)
  QK(hp+1) is interleaved into attention(hp) (qz/k2 double-buffered) so
  the PE fills the exp-wait slack; the output projection is fused into
  attention(hp3) per query block (delayed by one) to kill the tail.
"""
import numpy as np
from contextlib import ExitStack

import ml_dtypes

import concourse.bass as bass
import concourse.mybir as mybir
import concourse.tile as tile
from concourse import bacc
from concourse.bass_utils import run_bass_kernel_spmd

f32 = mybir.dt.float32
bf16 = mybir.dt.bfloat16
EXP = mybir.ActivationFunctionType.Exp
MULT = mybir.AluOpType.mult

B, N = 4, 2048
DIM = 1024
HL = 8          # heads per core
DH = 64
KD = DIM // 128  # 8 contraction chunks
NS = N // 128    # 16 key chunks
NQ = N // 512    # 4 query blocks
HP = HL // 2     # 4 head pairs


def build():
    nc = bacc.Bacc(None, target_bir_lowering=False, debug=False)
    xT = nc.declare_dram_parameter("xT", [DIM, N], bf16, isOutput=False)
    wq = nc.declare_dram_parameter("wq", [DIM, HL * DH], bf16, isOutput=False)
    wk = nc.declare_dram_parameter("wk", [DIM, HL * DH], bf16, isOutput=False)
    wv = nc.declare_dram_parameter("wv", [DIM, HL * DH], bf16, isOutput=False)
    wo = nc.declare_dram_parameter("wo", [HL * DH, DIM], bf16, isOutput=False)
    yT = nc.declare_dram_parameter("yT", [DIM, N], f32, isOutput=True)

    with tile.TileContext(nc) as tc, ExitStack() as ctx:
        p1 = ctx.enter_context(tc.tile_pool(name="p1", bufs=1))
        p2 = ctx.enter_context(tc.tile_pool(name="p2", bufs=2))
        psS = ctx.enter_context(tc.tile_pool(name="psS", bufs=2, space="PSUM"))
        psV = ctx.enter_context(tc.tile_pool(name="psV", bufs=4, space="PSUM"))

        # ---- parameter loads, spread across four engine DMA queues so the
        # transfers overlap (sync: x; scalar: wq/wk; vector: wv; gpsimd: wo)
        wqt, wkt, xt = [], [], []
        for k in range(KD):
            tq = p1.tile([128, HL * DH], bf16, tag=f"wq{k}", name=f"wq{k}")
            nc.gpsimd.dma_start(out=tq[:], in_=wq[k * 128:(k + 1) * 128, :])
            wqt.append(tq)
            tk = p1.tile([128, HL * DH], bf16, tag=f"wk{k}", name=f"wk{k}")
            nc.gpsimd.dma_start(out=tk[:], in_=wk[k * 128:(k + 1) * 128, :])
            wkt.append(tk)
            t = p1.tile([128, N], bf16, tag=f"xt{k}", name=f"xt{k}")
            nc.sync.dma_start(out=t[:], in_=xT[k * 128:(k + 1) * 128, :])
            xt.append(t)
        wvt = []
        for k in range(KD):
            tw = p1.tile([128, HL * DH], bf16, tag=f"wv{k}", name=f"wv{k}")
            nc.gpsimd.dma_start(out=tw[:], in_=wv[k * 128:(k + 1) * 128, :])
            wvt.append(tw)
        wot = []
        for j in range(HP):
            tw = p1.tile([128, DIM], bf16, tag=f"wo{j}", name=f"wo{j}")
            nc.gpsimd.dma_start(out=tw[:], in_=wo[j * 128:(j + 1) * 128, :])
            wot.append(tw)

        # ---- persistent attention tiles ----
        # qz/k2 double-buffered across head pairs
        qz = [[p1.tile([128, N], bf16, tag=f"qz{h}_{b}", name=f"qz{h}_{b}")
               for h in range(2)] for b in range(2)]
        k2 = [p1.tile([128, N], bf16, tag=f"k2_{b}", name=f"k2_{b}")
              for b in range(2)]
        for b in range(2):
            nc.vector.memset(qz[b][0][64:128, :].bitcast(f32), 0.0)
            nc.vector.memset(qz[b][1][0:64, :].bitcast(f32), 0.0)

        # per-head stride 66 (not 65) keeps every head's 128-col lhsT
        # window 4-byte aligned in bf16; cols 64 AND 65 are ones (col 64 is
        # the denominator row, 65 is junk M-row padding anyway)
        v2 = [p1.tile([128, HL * 66 + 64], bf16, tag=f"v2_{st}", name=f"v2_{st}")
              for st in range(NS)]
        ot = [p1.tile([128, N], bf16, tag=f"ot{j}", name=f"ot{j}")
              for j in range(HP)]

        def emit_qk(hp, on_scalar=False):
            """project Q^T and K^T for head pair hp into qz/k2 buffer hp%2;
            yields 4 times (once per quarter) so the caller can interleave.
            Casts go to ScalarE when interleaved with attention (the DVE
            queue there is on the AV critical path); nb01 before nb23 and
            q before k within each so the first scores unblock earliest."""
            buf = hp % 2
            for nb in ((0, 1), (2, 3)):
                for which, warr in (("q", wqt), ("k", wkt)):
                    pss = psS.tile([128, 1024], f32, tag="s", name="qkps")
                    for k in range(KD):
                        lhsT = warr[k][:, hp * 128:(hp + 1) * 128]
                        for i, n in enumerate(nb):
                            nc.tensor.matmul(
                                pss[:, i * 512:(i + 1) * 512], lhsT,
                                xt[k][:, n * 512:(n + 1) * 512],
                                start=(k == 0), stop=(k == KD - 1))
                    nsl = slice(nb[0] * 512, (nb[1] + 1) * 512)
                    if which == "q":
                        dsts = [(qz[buf][0][0:64, nsl], pss[0:64, :]),
                                (qz[buf][1][64:128, nsl], pss[64:128, :])]
                    else:
                        dsts = [(k2[buf][:, nsl], pss[:, :])]
                    for o, i_ in dsts:
                        if on_scalar:
                            nc.scalar.copy(o, i_)
                        else:
                            nc.vector.tensor_copy(out=o, in_=i_)
                    yield

        # f32 whose bits are two packed bf16 1.0s (for the paired ones cols)
        ONES2 = float(np.frombuffer(b"\x80\x3f\x80\x3f", "<f4")[0])

        def emit_v_pair(st):
            """project V chunks st, st+1 into one [128,1024] psS-pool tile
            (scores tag: rotates acyclically with the s tiles, unlike the
            pv tag whose buffers are held across a whole query block)."""
            vps = psS.tile([128, 1024], f32, tag="s", name="vps2")
            for half in range(2):
                v2t = v2[st + half]
                v3 = v2t[:, 0:HL * 66].rearrange("p (h c) -> p h c", h=HL)
                ones = v2t[:, 0:HL * 66].bitcast(f32).rearrange(
                    "p (h c) -> p h c", h=HL)  # [128, 8, 33] f32 view
                nc.vector.memset(ones[:, :, 32:33], ONES2)
                nc.vector.memset(v2t[:, HL * 66:].bitcast(f32), 0.0)
                vh = vps[:, half * 512:(half + 1) * 512]
                for k in range(KD):
                    nc.tensor.matmul(
                        vh, xt[k][:, (st + half) * 128:(st + half + 1) * 128],
                        wvt[k][:], start=(k == 0), stop=(k == KD - 1))
                nc.vector.tensor_copy(
                    out=v3[:, :, 0:64],
                    in_=vh.rearrange("p (h d) -> p h d", h=HL))

        def emit_outproj(n):
            """output projection for query block n (needs all ot)."""
            for dt in range(KD):
                yps = psV.tile([128, 512], f32, tag="pv", name="yps")
                for j in range(HP):
                    nc.tensor.matmul(yps[:], wot[j][:, dt * 128:(dt + 1) * 128],
                                     ot[j][:, n * 512:(n + 1) * 512],
                                     start=(j == 0), stop=(j == HP - 1))
                ysb = p2.tile([128, 512], f32, tag="y", name="ysb")
                nc.vector.tensor_copy(out=ysb[:], in_=yps[:])
                nc.sync.dma_start(out=yT[dt * 128:(dt + 1) * 128,
                                         n * 512:(n + 1) * 512], in_=ysb[:])

        # Schraudolph exp on the DVE: bf16 bits of exp(x) ~= int16 round of
        # x*(log2e*128) + (127 - sigma)*128, computed as one fused
        # tensor_scalar (mult, add) with int16 convert-on-write, then the
        # int16 tile is bitcast to bf16.  Max rel ripple ~3%; applied to a
        # minority of tiles to offload the saturated ScalarE.
        SCH_A = float(0.125 * np.log2(np.e) * 128.0)
        # +0.5: the HW f32->int16 convert truncates (CoreSim rounds)
        SCH_B = float((127.0 - 0.0430) * 128.0 + 0.5)
        ADD = mybir.AluOpType.add
        i16 = mybir.dt.int16

        def emit_attention(hp, filler, step_hook=None, dve_steps=()):
            """attention for head pair hp; `filler` is a list of callables
            (one per query block) emitted after each block to fill the PE
            while ScalarE catches up on the exp backlog.  step_hook(qb, ms)
            is emitted inside the ms loop; h01=1 exps of ms-steps listed in
            dve_steps run on the DVE (Schraudolph) instead of ScalarE."""
            buf = hp % 2
            kk, qq = k2[buf], qz[buf]
            sps = {}

            def emit_s2(qb, ms):
                qsl = slice(qb * 512, (qb + 1) * 512)
                s = [psS.tile([128, 1024], f32, tag="s", name="s")
                     for _ in range(2)]
                for i in range(2):     # k-chunk ms+i (shared ldweights)
                    for h01 in range(2):
                        nc.tensor.matmul(
                            s[h01][:, i * 512:(i + 1) * 512],
                            kk[:, (ms + i) * 128:(ms + i + 1) * 128],
                            qq[h01][:, qsl], start=True, stop=True)
                sps[(qb, ms)] = s

            emit_s2(0, 0)
            for qb in range(NQ):
                qsl = slice(qb * 512, (qb + 1) * 512)
                pv = [psV.tile([128, 512], f32, tag="pv", name="pv")
                      for _ in range(2)]
                for ms in range(0, NS, 2):
                    if step_hook is not None:
                        step_hook(qb, ms)
                    if ms + 2 < NS:
                        emit_s2(qb, ms + 2)
                    elif qb + 1 < NQ:
                        emit_s2(qb + 1, 0)   # cross-block prefetch
                    s = sps.pop((qb, ms))
                    for h01 in range(2):
                        if h01 == 1 and (ms // 2) in dve_steps:
                            pti = p2.tile([128, 1024], i16, tag="pti",
                                          name="pti", bufs=2)
                            nc.vector.tensor_scalar(
                                out=pti[:], in0=s[h01][:],
                                scalar1=SCH_A, scalar2=SCH_B,
                                op0=MULT, op1=ADD)
                            pt = pti.bitcast(bf16)
                        else:
                            pt = p2.tile([128, 1024], bf16, tag="pt",
                                         name="pt", bufs=4)
                            nc.scalar.activation(pt[:], s[h01][:], EXP,
                                                 scale=0.125)
                        l = hp * 2 + h01
                        nc.tensor.matmul(pv[h01][:],
                                         v2[ms][:, l * 66:l * 66 + 128],
                                         pt[:, 0:512],
                                         start=(ms == 0), stop=False)
                        nc.tensor.matmul(pv[h01][:],
                                         v2[ms + 1][:, l * 66:l * 66 + 128],
                                         pt[:, 512:1024],
                                         start=False, stop=(ms + 2 == NS))
                # normalize + evict: denominator row to SBUF partition 0,
                # reciprocal, broadcast across partitions, scale the O rows.
                srowt = p1.tile([1, 1024], f32, tag="srow", name="srow", bufs=2)
                rsum = p1.tile([1, 1024], f32, tag="rsum", name="rsum", bufs=2)
                for h01 in range(2):
                    srow = srowt[:, h01 * 512:(h01 + 1) * 512]
                    nc.vector.tensor_copy(out=srow, in_=pv[h01][64:65, :])
                    rs = rsum[:, h01 * 512:(h01 + 1) * 512]
                    nc.vector.reciprocal_approx_fast(out=rs, in_=srow)
                    rb = p1.tile([64, 512], f32, tag=f"rb{h01}", name="rb",
                                 bufs=2)
                    nc.gpsimd.partition_broadcast(rb[:], rs)
                    lo = h01 * 64
                    nc.vector.tensor_tensor(out=ot[hp][lo:lo + 64, qsl],
                                            in0=pv[h01][0:64, :],
                                            in1=rb[:], op=MULT)
                if filler and qb < len(filler):
                    filler[qb]()

        # ---- program order ----
        qk0 = list(emit_qk(0))  # generator fully drained: QK(hp0) up front
        del qk0
        emit_v_pair(0)

        def v_hook(qb, ms):
            # hp0 qb0: V projection chunks just-in-time, two per ms step
            if qb == 0 and ms + 2 < NS:
                emit_v_pair(ms + 2)

        DVE_STEPS = (1, 4, 6)   # 3 of 16 exps per query block on the DVE
        for hp in range(HP):
            if hp + 1 < HP:
                g = emit_qk(hp + 1, on_scalar=True)
                filler = [lambda g=g: next(g, None) for _ in range(NQ)]
            else:
                # last head pair: fuse the output projection, delayed by
                # one query block so the normalize chain is never waited on
                filler = [lambda n=n: emit_outproj(n) if n >= 0 else None
                          for n in (-1, 0, 1, 2)]
            emit_attention(hp, filler,
                           step_hook=v_hook if hp == 0 else None,
                           dve_steps=DVE_STEPS)
        emit_outproj(NQ - 1)

    nc.finalize()
    return nc


def make_in_map(x_b, w_qkv, w_out, g):
    cols = slice(g * 512, (g + 1) * 512)
    b16 = ml_dtypes.bfloat16
    return {
        "xT": np.ascontiguousarray(x_b.T).astype(b16),
        "wq": np.ascontiguousarray(w_qkv[:, 0:1024][:, cols]).astype(b16),
        "wk": np.ascontiguousarray(w_qkv[:, 1024:2048][:, cols]).astype(b16),
        "wv": np.ascontiguousarray(w_qkv[:, 2048:3072][:, cols]).astype(b16),
        "wo": np.ascontiguousarray(w_out[cols, :]).astype(b16),
    }


_NC_CACHE = {}


def _get_nc():
    if "nc" not in _NC_CACHE:
        _NC_CACHE["nc"] = build()
    return _NC_CACHE["nc"]


def kernel(x, w_qkv, w_out, b_out, trace=False):
    x = np.ascontiguousarray(np.asarray(x, dtype=np.float32))
    w_qkv = np.ascontiguousarray(np.asarray(w_qkv, dtype=np.float32))
    w_out = np.ascontiguousarray(np.asarray(w_out, dtype=np.float32))
    b_out = np.asarray(b_out, dtype=np.float32)

    nc = _get_nc()
    in_maps = [make_in_map(x[c // 2], w_qkv, w_out, c % 2) for c in range(8)]
    r = run_bass_kernel_spmd(nc, in_maps, list(range(8)), trace=trace)
    _NC_CACHE["exec_time_ns"] = r.exec_time_ns

    out = np.empty((B, N, DIM), np.float32)
    for b in range(B):
        out[b] = (r.results[2 * b]["yT"] + r.results[2 * b + 1]["yT"]).T + b_out
    return out
